# revision 18
# baseline (speedup 1.0000x reference)
"""MoE layer (8 experts, top-2) on 8 TRN2 NeuronCores — expert parallelism.

Contract: kernel(**inputs) takes FULL inputs, returns FULL output.
Strategy:
  - Host computes the (tiny) gate: logits -> top-2 -> softmax, gathers tokens
    per expert (dispatch), and scatter-adds the scaled expert outputs back
    (combine).  Gate probs are applied on the host during the combine, so the
    device kernel is a plain per-expert FFN.
  - Capacity balancing: per-core columns C are split [0, a) + [a, C).  The A
    range runs the core's own expert; the B range runs spill-over tokens of an
    overloaded expert (second weight set), so C ~ max(1024, fragmentation)
    instead of the max expert load.  (C, a) come from a tiny search over the
    actual expert loads; single-set fallback when no packing helps.
  - Core kernel (bf16 operands, f32 psum): both stages stream the token dim
    as the matmul moving dimension, so PE cost is 512*C cycles exactly:
      phase 1: hT[f, t] = relu(w1.T @ xgt + b1)   (256*C PE cycles)
      phase 2: yT[d, t] = w2-tiles.T @ hT         (256*C PE cycles)
    hT ([128, 32, C] bf16) stays SBUF-resident; w1 streams per F-block in
    phase 1; w2 streams per D-tile in phase 2 from a host-preblocked layout
    ([dt, fi, fo*di]) so every DMA line is 8 KiB.
  - A PE "warmup" chain of dummy matmuls (on a memset tile, no DMA deps)
    runs the p-state clock ramp during the head DMA.

Shapes (hardcoded from the problem spec):
  x [2048, 2, 1024], gate_w [1024, 8], gate_b [8],
  w1 [8, 1024, 4096], b1 [8, 4096], w2 [8, 4096, 1024], b2 [8, 1024].
"""
import sys
import numpy as np

for _p in ("/opt/trn_rl_repo", "/root/.axon_site/_ro/trn_rl_repo"):
    if _p not in sys.path:
        sys.path.insert(0, _p)

import ml_dtypes
import concourse.bacc as bacc
import concourse.tile as tile
import concourse.mybir as mybir
from concourse import bass2jax, mybir as _mybir

N_EXPERTS = 8
TOP_K = 2
S, B, D, F = 2048, 2, 1024, 4096
P = 128
FB = 512                # F-block size streamed through SBUF in phase 1
NB = F // FB            # 8 F-blocks
FC = FB // P            # 4 F-partition-tiles per block
FO = F // P             # 32 F-partition-tiles total
DK = D // P             # 8 contraction tiles for phase 1
DT = D // P             # 8 output D-tiles for phase 2

_f32 = mybir.dt.float32
_bf16 = mybir.dt.bfloat16
_bf16_np = ml_dtypes.bfloat16

_NC_CACHE: dict = {}
_C_MAX = 1664           # max capacity per pass (SBUF budget bound)
LAST_DEVICE_NS = -1     # wall-clock of the last device dispatch (incl. transfers)
LAST_C = -1
LAST_A = -1


def _c_chunks(C):
    """Split C into chunks <=512 (PSUM bank width in f32), remainder last."""
    out, pos = [], 0
    while C - pos >= 512:
        out.append((pos, 512))
        pos += 512
    if C - pos:
        out.append((pos, C - pos))
    return out


def _pack(loads):
    """Pick (C, a): per-core columns C, own-expert range [0, a), spill range
    [a, C).  Feasible iff the per-expert surpluses over `a` fit in 8 spill
    slots of size C-a (<=512, one per core).  Falls back to single-set."""
    mx = max(loads)
    lo = -(-sum(loads) // len(loads))
    best = None
    for C in range(-(-lo // 4) * 4, mx + 1, 4):
        for a in range(max(C - 512, 1), C):
            bsz = C - a
            need = sum(-(-max(0, n - a) // bsz) for n in loads)
            if need <= len(loads):
                best = (C, a)
                break
        if best:
            break
    if best is None or best[0] >= mx:
        C = -(-mx // 4) * 4
        return C, C
    return best


def _build(C, a, passes=1, *, psum_bufs=8, w1_bufs=2, w2_bufs=4, y_bufs=2,
           n_warm=30, warm_w=128):
    """Trace + compile the per-core SPMD program.

    Columns [0, a) use weight set A, [a, C) weight set B (skipped if a == C).
    passes>1 repeats the whole compute (same output) — used only for
    differential timing of the device kernel.
    """
    key = (C, a, passes, psum_bufs, w1_bufs, w2_bufs, y_bufs, n_warm, warm_w)
    if key in _NC_CACHE:
        return _NC_CACHE[key]
    dual = a < C
    nc = bacc.Bacc("TRN2", target_bir_lowering=False, debug=False,
                   enable_asserts=False, num_devices=8)
    xgt_d = nc.dram_tensor("xgt", (D, C), _bf16, kind="ExternalInput").ap()
    w1a_d = nc.dram_tensor("w1a", (D, F), _bf16, kind="ExternalInput").ap()
    b1a_d = nc.dram_tensor("b1a", (P, FO), _f32, kind="ExternalInput").ap()
    # w2 host-preblocked: [dt*fi, fo*di] so per-(dt) DMA lines are 8 KiB
    w2a_d = nc.dram_tensor("w2a", (DT * P, FO * P), _bf16, kind="ExternalInput").ap()
    if dual:
        w1b_d = nc.dram_tensor("w1b", (D, F), _bf16, kind="ExternalInput").ap()
        b1b_d = nc.dram_tensor("b1b", (P, FO), _f32, kind="ExternalInput").ap()
        w2b_d = nc.dram_tensor("w2b", (DT * P, FO * P), _bf16, kind="ExternalInput").ap()
    yT_d = nc.dram_tensor("yT", (D, C), _f32, kind="ExternalOutput").ap()

    xgt_r = xgt_d.rearrange("(ko ki) c -> ki ko c", ki=P)      # [128, 8, C]
    w1a_r = w1a_d.rearrange("(ko ki) f -> ki ko f", ki=P)      # [128, 8, F]
    w2a_r = w2a_d.rearrange("(dt fi) fod -> fi dt fod", fi=P)  # [128, 8, 4096]
    if dual:
        w1b_r = w1b_d.rearrange("(ko ki) f -> ki ko f", ki=P)
        w2b_r = w2b_d.rearrange("(dt fi) fod -> fi dt fod", fi=P)
    yT_r = yT_d.rearrange("(do di) c -> di do c", di=P)        # [128, 8, C]

    chunksA = _c_chunks(a)
    chunks = [(cs, csz, 0) for (cs, csz) in chunksA]
    if dual:
        chunks.append((a, C - a, 1))

    with tile.TileContext(nc) as tc:
        with tc.tile_pool(name="const", bufs=1) as cpool, \
             tc.tile_pool(name="w1p", bufs=w1_bufs) as w1pool, \
             tc.tile_pool(name="w2p", bufs=w2_bufs) as w2pool, \
             tc.tile_pool(name="yp", bufs=y_bufs) as ypool, \
             tc.tile_pool(name="ps", bufs=psum_bufs, space="PSUM") as psum:
            xgt_sb = cpool.tile([P, DK, C], _bf16)
            b1_sb = [cpool.tile([P, FO], _f32, name=f"b1_{i}")
                     for i in range(1 + dual)]
            # separate A/B hT tiles: phase-2 A groups must not pick up a
            # whole-tile dependency on the late B relus
            hT_g = [cpool.tile([P, FO, a], _bf16, name="hTA")]
            if dual:
                hT_g.append(cpool.tile([P, FO, C - a], _bf16, name="hTB"))

            if n_warm:
                # PE p-state warmup: memset a tile (no DMA dependency) and
                # chain dummy matmuls so the clock ramp runs concurrently
                # with the head DMA.  Result is never read.
                warm = cpool.tile([P, warm_w], _bf16)
                nc.vector.memset(warm[:], 0.0)
                wps = psum.tile([P, warm_w], _f32, tag="ps", name="warm_ps")
                for i in range(n_warm):
                    nc.tensor.matmul(wps[:], warm[:], warm[:],
                                     start=(i == 0), stop=(i == n_warm - 1))

            def relu(fb, fc, ch, ps):
                cs, csz, g = ch
                hcs = cs - (a if g else 0)
                fcol = fb * FC + fc
                nc.scalar.activation(
                    hT_g[g][:, fcol, hcs:hcs + csz], ps[:, :csz],
                    mybir.ActivationFunctionType.Relu,
                    bias=b1_sb[g][:, fcol:fcol + 1], scale=1.0,
                )

            def stage1_groups(fb, w1_t, use_chunks, wave_head=False):
                groups = [(fc, ch) for ch in use_chunks for fc in range(FC)]
                if wave_head:
                    # dk-major waves: up to psum_bufs groups accumulate
                    # concurrently so the PE consumes each xgt[dk] as it lands
                    for ws in range(0, len(groups), psum_bufs):
                        wave = groups[ws:ws + psum_bufs]
                        pss = [psum.tile([P, 512], _f32, name=f"ps1h_{ws}_{i}",
                                         tag="ps")
                               for i in range(len(wave))]
                        for dk in range(DK):
                            for (fc, ch), ps in zip(wave, pss):
                                cs, csz, g = ch
                                nc.tensor.matmul(
                                    ps[:, :csz],
                                    w1_t[g][:, dk, fc * P:(fc + 1) * P],
                                    xgt_sb[:, dk, cs:cs + csz],
                                    start=(dk == 0), stop=(dk == DK - 1),
                                )
                        for (fc, ch), ps in zip(wave, pss):
                            relu(fb, fc, ch, ps)
                else:
                    for (fc, ch) in groups:
                        cs, csz, g = ch
                        ps = psum.tile([P, 512], _f32, tag="ps")
                        for dk in range(DK):
                            nc.tensor.matmul(
                                ps[:, :csz],
                                w1_t[g][:, dk, fc * P:(fc + 1) * P],
                                xgt_sb[:, dk, cs:cs + csz],
                                start=(dk == 0), stop=(dk == DK - 1),
                            )
                        relu(fb, fc, ch, ps)

            chunksA_g = [ch for ch in chunks if ch[2] == 0]
            chunksB_g = [ch for ch in chunks if ch[2] == 1]

            def phase1(first_rep):
                """B work is deferred one section: w1b(fb) loads ride behind
                w1a(fb+1), and B(fb) groups run after A(fb+1) — keeps the
                large w1b loads out of the congested head DMA window and
                gives each one a full A-section to land."""
                def emit_b(pfb):
                    w1b_t = w1pool.tile([P, DK, FB], _bf16, tag="w1b",
                                        name="w1b_t")
                    nc.sync.dma_start(w1b_t[:],
                                      w1b_r[:, :, pfb * FB:(pfb + 1) * FB])
                    return w1b_t

                for fb in range(NB):
                    w1a_t = w1pool.tile([P, DK, FB], _bf16, tag="w1a",
                                        name="w1a_t")
                    if first_rep and fb == 0:
                        # head: per-dk interleave of w1a-block0 and xgt so the
                        # PE starts on dk 0 while later dk slices stream in
                        for dk in range(DK):
                            nc.sync.dma_start(w1a_t[:, dk],
                                              w1a_r[:, dk, 0:FB])
                            nc.sync.dma_start(xgt_sb[:, dk], xgt_r[:, dk])
                        nc.sync.dma_start(b1_sb[0][:], b1a_d)
                        if dual:
                            nc.sync.dma_start(b1_sb[1][:], b1b_d)
                    else:
                        nc.sync.dma_start(w1a_t[:],
                                          w1a_r[:, :, fb * FB:(fb + 1) * FB])
                    if dual and fb >= 1:
                        w1b_t = emit_b(fb - 1)
                    stage1_groups(fb, [w1a_t], chunksA_g,
                                  wave_head=(first_rep and fb == 0))
                    if dual and fb >= 1:
                        stage1_groups(fb - 1, [None, w1b_t], chunksB_g)
                if dual:
                    w1b_t = emit_b(NB - 1)
                    stage1_groups(NB - 1, [None, w1b_t], chunksB_g)

            def phase2():
                """yT[dt, :] = sum_fo w2[fo, dt].T @ hT[fo, :], streamed out."""
                for dt in range(DT):
                    w2_t = [w2pool.tile([P, FO * P], _bf16, tag="w2a", name="w2a_t")]
                    nc.sync.dma_start(w2_t[0][:], w2a_r[:, dt, :])
                    if dual:
                        w2_t.append(w2pool.tile([P, FO * P], _bf16, tag="w2b", name="w2b_t"))
                        nc.sync.dma_start(w2_t[1][:], w2b_r[:, dt, :])
                    for (cs, csz, g) in chunks:
                        hcs = cs - (a if g else 0)
                        ps2 = psum.tile([P, 512], _f32, tag="ps")
                        for fo in range(FO):
                            nc.tensor.matmul(
                                ps2[:, :csz],
                                w2_t[g][:, fo * P:(fo + 1) * P],
                                hT_g[g][:, fo, hcs:hcs + csz],
                                start=(fo == 0), stop=(fo == FO - 1),
                            )
                        yt = ypool.tile([P, 512], _f32)
                        nc.vector.tensor_copy(yt[:, :csz], ps2[:, :csz])
                        nc.sync.dma_start(yT_r[:, dt, cs:cs + csz], yt[:, :csz])

            for rep in range(passes):
                phase1(rep == 0)
                phase2()
    nc.compile()
    _NC_CACHE[key] = nc
    return nc


class _Runner:
    """Persistent jitted SPMD executor for a compiled Bacc program.

    Mirrors bass2jax.run_bass_via_pjrt but keeps the jitted callable so
    repeat calls skip retracing/recompiling.
    """

    def __init__(self, nc, n_cores):
        import jax
        from jax.sharding import Mesh, PartitionSpec
        from jax.experimental.shard_map import shard_map

        bass2jax.install_neuronx_cc_hook()
        self.nc = nc
        self.n_cores = n_cores
        in_names, out_names, out_avals = [], [], []
        for alloc in nc.m.functions[0].allocations:
            if not isinstance(alloc, _mybir.MemoryLocationSet):
                continue
            name = alloc.memorylocations[0].name
            if alloc.kind == "ExternalInput":
                in_names.append(name)
            elif alloc.kind == "ExternalOutput":
                out_names.append(name)
                out_avals.append(jax.core.ShapedArray(
                    tuple(alloc.tensor_shape), _mybir.dt.np(alloc.dtype)))
        partition_name = nc.partition_id_tensor.name if nc.partition_id_tensor else None
        in_names = [n for n in in_names if n != partition_name]
        all_names = in_names + out_names + ([partition_name] if partition_name else [])
        self.in_names, self.out_names, self.out_avals = in_names, out_names, out_avals
        self._all_names, self._partition_name = all_names, partition_name
        n_params = len(in_names)

        def _body(*args):
            operands = list(args)
            if partition_name is not None:
                operands.append(bass2jax.partition_id_tensor())
            outs = bass2jax._bass_exec_p.bind(
                *operands,
                out_avals=tuple(out_avals),
                in_names=tuple(all_names),
                out_names=tuple(out_names),
                lowering_input_output_aliases=(),
                sim_require_finite=False,
                sim_require_nnan=False,
                nc=nc,
            )
            return tuple(outs)

        devices = jax.devices()[:n_cores]
        mesh = Mesh(np.asarray(devices), ("core",))
        n_outs = len(out_names)
        self._fn = jax.jit(
            shard_map(_body, mesh=mesh,
                      in_specs=(PartitionSpec("core"),) * (n_params + n_outs),
                      out_specs=(PartitionSpec("core"),) * n_outs,
                      check_rep=False),
            donate_argnums=tuple(range(n_params, n_params + n_outs)),
            keep_unused=True,
        )
        self._jax = jax

    def concat_inputs(self, in_maps):
        return [np.concatenate([np.asarray(m[name]) for m in in_maps], axis=0)
                for name in self.in_names]

    def zero_outs(self):
        jnp = self._jax.numpy
        return [jnp.zeros((self.n_cores * a.shape[0], *a.shape[1:]), a.dtype)
                for a in self.out_avals]

    def run_raw(self, concat_in, zouts):
        outs = self._fn(*concat_in, *zouts)
        self._jax.block_until_ready(outs)
        return outs

    def run(self, in_maps):
        outs = self.run_raw(self.concat_inputs(in_maps), self.zero_outs())
        return [
            {name: np.asarray(outs[i]).reshape(self.n_cores, *self.out_avals[i].shape)[c]
             for i, name in enumerate(self.out_names)}
            for c in range(self.n_cores)
        ]


_RUNNER_CACHE: dict = {}


def _runner(C, a, passes=1):
    key = (C, a, passes)
    if key not in _RUNNER_CACHE:
        _RUNNER_CACHE[key] = _Runner(_build(C, a, passes), N_EXPERTS)
    return _RUNNER_CACHE[key]


def _route(x2d, gate_w, gate_b):
    """Host gate: returns per-token top-2 expert ids and softmax probs (fp32)."""
    logits = x2d.astype(np.float64) @ gate_w.astype(np.float64) + gate_b.astype(np.float64)
    order = np.argsort(-logits, axis=-1, kind="stable")
    top2 = order[:, :TOP_K]                               # [T, 2]
    l = np.take_along_axis(logits, top2, axis=-1)         # [T, 2]
    m = l.max(axis=-1, keepdims=True)
    e = np.exp(l - m)
    p = (e / e.sum(axis=-1, keepdims=True)).astype(np.float32)
    return top2, p


def _block_w2(w2_e_bf):
    """[F, D] -> [dt*fi, fo*di] so per-dt DMA lines are contiguous 8 KiB."""
    return np.ascontiguousarray(
        w2_e_bf.reshape(FO, P, DT, P).transpose(2, 1, 0, 3).reshape(DT * P, FO * P))


def _b1t(b1_e):
    return np.ascontiguousarray(b1_e.astype(np.float32).reshape(FO, P).T)


def prepare(x, gate_w, gate_b, w1, b1, w2):
    """Routing + packing + per-core input maps.  Returns (in_maps, metas,
    C, a) where metas[c] = (ix_a, p_a, n_a, ix_b, p_b, n_b)."""
    T = S * B
    x2d = np.ascontiguousarray(np.asarray(x, np.float32).reshape(T, D))
    top2, p = _route(x2d, np.asarray(gate_w, np.float32),
                     np.asarray(gate_b, np.float32))
    idx_lists = []
    for e in range(N_EXPERTS):
        sel = np.nonzero(top2 == e)          # (token_idx, slot_idx)
        idx_lists.append((sel[0], p[sel[0], sel[1]]))
    loads = [len(ix) for ix, _ in idx_lists]
    C, a = _pack(loads)

    # spill assignment: surplus tokens of overloaded experts -> one slice of
    # size <= C-a per core (single-set mode when a == C has no spill)
    spill = [None] * N_EXPERTS               # per core: (expert, lo, hi)
    if a < C:
        bsz = C - a
        pieces = []
        for e in range(N_EXPERTS):
            n = loads[e]
            for lo in range(a, n, bsz):
                pieces.append((e, lo, min(lo + bsz, n)))
        assert len(pieces) <= N_EXPERTS
        free = [c for c in range(N_EXPERTS)]
        # prefer a core whose own expert spilled (keeps its weights identical)
        for pc in pieces:
            e = pc[0]
            c = e if e in free else free[0]
            free.remove(c)
            spill[c] = pc

    xT_bf = np.ascontiguousarray(x2d.T.astype(_bf16_np))  # [D, T] bf16
    w1_bf = [np.ascontiguousarray(np.asarray(w1[e]).astype(_bf16_np))
             for e in range(N_EXPERTS)]
    w2_blk = [_block_w2(np.asarray(w2[e]).astype(_bf16_np))
              for e in range(N_EXPERTS)]
    b1_t = [_b1t(np.asarray(b1[e])) for e in range(N_EXPERTS)]

    in_maps, metas = [], []
    for c in range(N_EXPERTS):
        ix_a, p_a = idx_lists[c]
        ix_a, p_a = ix_a[:a], p_a[:a]
        n_a = len(ix_a)
        xgt = np.zeros((D, C), dtype=_bf16_np)
        xgt[:, :n_a] = xT_bf[:, ix_a]
        m = {"xgt": xgt, "w1a": w1_bf[c], "b1a": b1_t[c], "w2a": w2_blk[c]}
        ix_b = p_b = None
        n_b = 0
        if a < C:
            be = spill[c][0] if spill[c] else c
            if spill[c]:
                e, lo, hi = spill[c]
                ix_b, p_b = idx_lists[e][0][lo:hi], idx_lists[e][1][lo:hi]
                n_b = hi - lo
                xgt[:, a:a + n_b] = xT_bf[:, ix_b]
            m.update({"w1b": w1_bf[be], "b1b": b1_t[be], "w2b": w2_blk[be]})
        in_maps.append(m)
        metas.append((ix_a, p_a, n_a, ix_b, p_b, n_b))
    return in_maps, metas, C, a


def kernel(x, gate_w, gate_b, w1, b1, w2, b2):
    in_maps, metas, C, a = prepare(x, gate_w, gate_b, w1, b1, w2)
    global LAST_C, LAST_A
    LAST_C, LAST_A = C, a
    runner = _runner(C, a)

    import time as _time
    _t0 = _time.time()
    results = runner.run(in_maps)
    global LAST_DEVICE_NS
    LAST_DEVICE_NS = int((_time.time() - _t0) * 1e9)

    T = S * B
    out2d = np.zeros((T, D), dtype=np.float32)
    for c in range(N_EXPERTS):
        ix_a, p_a, n_a, ix_b, p_b, n_b = metas[c]
        yT = results[c]["yT"]
        if n_a:
            # combine: scale by gate prob during the scatter-add
            out2d[ix_a] += p_a[:, None] * yT[:, :n_a].T
        if n_b:
            out2d[ix_b] += p_b[:, None] * yT[:, a:a + n_b].T

    b2 = np.asarray(b2, np.float32)
    if np.any(b2):
        x2d = np.asarray(x, np.float32).reshape(T, D)
        top2, p = _route(x2d, np.asarray(gate_w, np.float32),
                         np.asarray(gate_b, np.float32))
        comb = np.zeros((T, N_EXPERTS), dtype=np.float32)
        np.put_along_axis(comb, top2, p, axis=-1)
        out2d += comb @ b2
    return out2d.reshape(S, B, D)
